# revision 8
# baseline (speedup 1.0000x reference)
"""Multi-head self-attention (batch=2, seq=2048, embed=1024, heads=16, causal)
sharded over 8 NeuronCores: data-parallel over batch (x2) and tensor-parallel
over heads (x4 groups of 4 heads).

Each core computes, for its (batch b, head group g):
  qkvT proj (transposed activations), causal softmax attention with the
  denominator folded into the AV matmul via a ones-column on V, and a partial
  output projection W_out[:, cols_g].T @ o_hat in transposed layout.
Host sums the 4 partials per batch, transposes back, and adds the constant
row  b_out + W_out @ b_v  (exact bias algebra).
"""

import os

import numpy as np
from contextlib import ExitStack

import concourse.bass as bass
import concourse.mybir as mybir
import concourse.tile as tile
from concourse import bacc
from concourse.bass_utils import run_bass_kernel_spmd

N_HEADS = 16
EMBED = 1024
HEAD = 64
SEQ = 2048
BATCH = 2
N_CORES = 8
HPC = 4                # heads per core
GCOLS = HPC * HEAD     # 256 embed columns per head group
P = 128
CH = 512               # seq chunk
NCH = SEQ // CH        # 4
KT = SEQ // P          # 16 k tiles

DT = mybir.dt.float32
DTR = mybir.dt.float32r

LAST_EXEC_NS = None
LAST_RESULTS = None


def _build_program():
    nc = bacc.Bacc("TRN2", target_bir_lowering=False, debug=False,
                   num_devices=N_CORES)
    xT = nc.dram_tensor("xT", [EMBED, SEQ], DTR, kind="ExternalInput")
    wqkT = nc.dram_tensor("wqkT", [EMBED, 2 * GCOLS], DTR, kind="ExternalInput")
    wvT = nc.dram_tensor("wvT", [EMBED, GCOLS], DTR, kind="ExternalInput")
    bqk = nc.dram_tensor("bqk", [P, 4], DT, kind="ExternalInput")
    woT = nc.dram_tensor("woT", [GCOLS, EMBED], DTR, kind="ExternalInput")
    maskT = nc.dram_tensor("maskT", [P, 4 * CH], DTR, kind="ExternalInput")
    onesc = nc.dram_tensor("onesc", [P, HPC], DTR, kind="ExternalInput")
    yT = nc.dram_tensor("yT", [EMBED, SEQ], DT, kind="ExternalOutput")

    with tile.TileContext(nc) as tc, ExitStack() as ctx:
        const = ctx.enter_context(tc.tile_pool(name="const", bufs=1))
        xpool = ctx.enter_context(tc.tile_pool(name="xpool", bufs=10))
        stpool = ctx.enter_context(tc.tile_pool(name="stpool", bufs=6))
        small = ctx.enter_context(tc.tile_pool(name="small", bufs=4))
        outsb = ctx.enter_context(tc.tile_pool(name="outsb", bufs=3))
        ps512 = ctx.enter_context(tc.tile_pool(name="ps512", bufs=5, space="PSUM"))
        psO = ctx.enter_context(tc.tile_pool(name="psO", bufs=2, space="PSUM"))

        # ---- persistent SBUF residents ----
        wqk_t = []
        wv_t = []
        for i in range(8):
            w1 = const.tile([P, 2 * GCOLS], DTR, tag=f"wqk{i}", name=f"wqk{i}")
            nc.sync.dma_start(out=w1, in_=wqkT[P * i:P * (i + 1), :])
            wqk_t.append(w1)
            w2 = const.tile([P, GCOLS], DTR, tag=f"wv{i}", name=f"wv{i}")
            nc.sync.dma_start(out=w2, in_=wvT[P * i:P * (i + 1), :])
            wv_t.append(w2)
        wo_t = []
        for k in range(2):
            w3 = const.tile([P, EMBED], DTR, tag=f"wo{k}", name=f"wo{k}")
            nc.sync.dma_start(out=w3, in_=woT[P * k:P * (k + 1), :])
            wo_t.append(w3)
        bqk_sb = const.tile([P, 4], DT, tag="bqk")
        nc.sync.dma_start(out=bqk_sb, in_=bqk[:])
        mask_sb = const.tile([P, 4 * CH], DTR, tag="mask")
        nc.sync.dma_start(out=mask_sb, in_=maskT[:])

        qt_t = [const.tile([P, SEQ], DTR, tag=f"qt{a}", name=f"qt{a}") for a in range(2)]
        kt_t = [const.tile([P, SEQ], DTR, tag=f"kt{a}", name=f"kt{a}") for a in range(2)]
        # v tiles: [128, 4 heads x 65] with a ones column per head at col 64
        vt_t = [const.tile([P, HPC * (HEAD + 1)], DTR, tag=f"vt{t}", name=f"vt{t}")
                for t in range(KT)]
        for t in range(KT):
            dst = vt_t[t].rearrange("p (h d) -> p h d", h=HPC)[:, :, HEAD:HEAD + 1]
            nc.sync.dma_start(out=dst, in_=onesc.rearrange("p (h o) -> p h o", o=1))
        ohat_t = [const.tile([P, SEQ], DTR, tag=f"ohat{a}", name=f"ohat{a}") for a in range(2)]

        for qi in range(NCH):
            sl = bass.ds(CH * qi, CH)
            # ---- QKV projection for this seq chunk ----
            xts = []
            for i in range(8):
                xt = xpool.tile([P, CH], DTR, tag="xt", name="xt")
                nc.sync.dma_start(out=xt, in_=xT[P * i:P * (i + 1), sl])
                xts.append(xt)
            for f in range(4):
                ps = ps512.tile([P, CH], DT, tag="ps512", name="ps512t")
                for i in range(8):
                    nc.tensor.matmul(
                        ps,
                        lhsT=wqk_t[i][:, bass.ds(P * f, P)],
                        rhs=xts[i][:],
                        start=(i == 0), stop=(i == 7),
                    )
                dst = qt_t[f] if f < 2 else kt_t[f - 2]
                nc.vector.tensor_scalar_add(dst[:, sl], ps, bqk_sb[:, f:f + 1])
            for s in range(4):
                ti = 4 * qi + s
                ps = ps512.tile([P, GCOLS], DT, tag="ps512", name="ps512v")
                for i in range(8):
                    nc.tensor.matmul(
                        ps,
                        lhsT=xts[i][:, bass.ds(P * s, P)],
                        rhs=wv_t[i][:],
                        start=(i == 0), stop=(i == 7),
                    )
                dst = vt_t[ti].rearrange("p (h d) -> p h d", h=HPC)[:, :, 0:HEAD]
                src = ps.rearrange("p (h d) -> p h d", h=HPC)
                nc.vector.tensor_copy(dst, src)

            # ---- attention for all 4 heads on this q chunk ----
            nk = 4 * qi + 4
            for hp in range(2):
                po = [psO.tile([HEAD + 1, CH], DT, tag="psO", name="psO") for _ in range(2)]
                for ki in range(nk):
                    sts = []
                    for hh in range(2):
                        h = 2 * hp + hh
                        r0 = HEAD * hh
                        ps = ps512.tile([P, CH], DT, tag="ps512", name="ps512t")
                        nc.tensor.matmul(
                            ps,
                            lhsT=kt_t[hp][r0:r0 + HEAD,
                                          bass.ds(P * ki, P)],
                            rhs=qt_t[hp][r0:r0 + HEAD, sl],
                            start=True, stop=True,
                        )
                        st = stpool.tile([P, CH], DTR, tag="st", name="st")
                        nc.scalar.activation(st[:], ps,
                                             mybir.ActivationFunctionType.Exp,
                                             scale=0.125)
                        kr = ki - 4 * qi
                        if kr >= 0:
                            nc.vector.tensor_mul(
                                st[:], st[:], mask_sb[:, bass.ds(CH * kr, CH)])
                        sts.append(st)
                    for hh in range(2):
                        h = 2 * hp + hh
                        nc.tensor.matmul(
                            po[hh],
                            lhsT=vt_t[ki][:, bass.ds((HEAD + 1) * h,
                                                     HEAD + 1)],
                            rhs=sts[hh][:],
                            start=(ki == 0), stop=(ki == nk - 1),
                        )
                for hh in range(2):
                    r0 = HEAD * hh
                    recip = small.tile([1, CH], DT, tag="recip", name="recip")
                    nc.vector.reciprocal(recip[:], po[hh][HEAD:HEAD + 1, :])
                    recipb = small.tile([HEAD, CH], DT, tag="recipb", name="recipb")
                    nc.gpsimd.partition_broadcast(recipb[:], recip[:])
                    nc.vector.tensor_mul(ohat_t[hp][r0:r0 + HEAD, sl],
                                         po[hh][0:HEAD, :], recipb[:])

            # ---- partial output projection for this seq chunk ----
            for m in range(8):
                ps = ps512.tile([P, CH], DT, tag="ps512", name="ps512t")
                for k in range(2):
                    nc.tensor.matmul(
                        ps,
                        lhsT=wo_t[k][:, bass.ds(P * m, P)],
                        rhs=ohat_t[k][:, sl],
                        start=(k == 0), stop=(k == 1),
                    )
                ot = outsb.tile([P, CH], DT, tag="ot", name="ot")
                nc.vector.tensor_copy(ot[:], ps)
                nc.sync.dma_start(out=yT[P * m:P * (m + 1), sl], in_=ot[:])

    nc.compile()
    return nc


def _make_masks():
    m = np.zeros((P, 4 * CH), dtype=np.float32)
    p = np.arange(P)[:, None]
    c = np.arange(CH)[None, :]
    for d in range(4):
        m[:, CH * d:CH * (d + 1)] = ((p + P * d) <= c).astype(np.float32)
    return m


def kernel(x, W_qkv, b_qkv, W_out, b_out):
    global LAST_EXEC_NS, LAST_RESULTS
    x = np.asarray(x, dtype=np.float32)
    W_qkv = np.asarray(W_qkv, dtype=np.float32)
    b_qkv = np.asarray(b_qkv, dtype=np.float32)
    W_out = np.asarray(W_out, dtype=np.float32)
    b_out = np.asarray(b_out, dtype=np.float32)

    nc = _build_program()
    masks = _make_masks()

    in_maps = []
    for c in range(N_CORES):
        b, g = divmod(c, HPC)
        q0 = GCOLS * g
        wq = W_qkv[q0:q0 + GCOLS]                    # [256, 1024]
        wk = W_qkv[EMBED + q0:EMBED + q0 + GCOLS]
        wv = W_qkv[2 * EMBED + q0:2 * EMBED + q0 + GCOLS]
        bq = b_qkv[q0:q0 + GCOLS]
        bk = b_qkv[EMBED + q0:EMBED + q0 + GCOLS]
        bqk = np.stack([bq[0:P], bq[P:2 * P], bk[0:P], bk[P:2 * P]],
                       axis=1).astype(np.float32)   # [128, 4]
        in_maps.append({
            "xT": np.ascontiguousarray(x[b].T),
            "wqkT": np.ascontiguousarray(np.concatenate([wq, wk], 0).T),
            "wvT": np.ascontiguousarray(wv.T),
            "bqk": np.ascontiguousarray(bqk),
            "woT": np.ascontiguousarray(W_out[:, q0:q0 + GCOLS].T),
            "maskT": masks,
            "onesc": np.ones((P, HPC), dtype=np.float32),
        })

    want_trace = bool(int(os.environ.get("KTRACE", "0")))
    if want_trace:
        try:
            import antenv.axon_hooks  # noqa: F401
        except ImportError:
            want_trace = False
    res = run_bass_kernel_spmd(nc, in_maps, list(range(N_CORES)),
                               trace=want_trace,
                               tmpdir=os.environ.get("KTRACE_DIR") or None)
    LAST_EXEC_NS = res.exec_time_ns
    LAST_RESULTS = res

    out = np.empty((BATCH, SEQ, EMBED), dtype=np.float32)
    crow = (b_out + W_out @ b_qkv[2 * EMBED:]).astype(np.float32)
    for b in range(BATCH):
        acc = np.zeros((EMBED, SEQ), dtype=np.float32)
        for g in range(HPC):
            acc += res.results[HPC * b + g]["yT"]
        out[b] = acc.T + crow[None, :]
    return out


# revision 11
# speedup vs baseline: 1.3198x; 1.3198x over previous
"""Multi-head self-attention (batch=2, seq=2048, embed=1024, heads=16, causal)
sharded over 8 NeuronCores: data-parallel over batch (x2) and tensor-parallel
over heads (x4 groups of 4 heads).

Each core computes, for its (batch b, head group g):
  qkvT proj (transposed activations), causal softmax attention with the
  denominator folded into the AV matmul via a ones-column on V, and a partial
  output projection W_out[:, cols_g].T @ o_hat in transposed layout.
Host sums the 4 partials per batch, transposes back, and adds the constant
row  b_out + W_out @ b_v  (exact bias algebra).
"""

import os

import ml_dtypes
import numpy as np
from contextlib import ExitStack

import concourse.bass as bass
import concourse.mybir as mybir
import concourse.tile as tile
from concourse import bacc
from concourse.bass_utils import run_bass_kernel_spmd

N_HEADS = 16
EMBED = 1024
HEAD = 64
SEQ = 2048
BATCH = 2
N_CORES = 8
HPC = 4                # heads per core
GCOLS = HPC * HEAD     # 256 embed columns per head group
P = 128
CH = 512               # seq chunk
NCH = SEQ // CH        # 4
KT = SEQ // P          # 16 k tiles

DT = mybir.dt.float32
DTB = mybir.dt.bfloat16

LAST_EXEC_NS = None
LAST_RESULTS = None


def _build_program():
    nc = bacc.Bacc("TRN2", target_bir_lowering=False, debug=False,
                   num_devices=N_CORES)
    xT = nc.dram_tensor("xT", [EMBED, SEQ], DTB, kind="ExternalInput")
    wqkT = nc.dram_tensor("wqkT", [EMBED, 2 * GCOLS], DTB, kind="ExternalInput")
    wvT = nc.dram_tensor("wvT", [EMBED, GCOLS], DTB, kind="ExternalInput")
    bqk = nc.dram_tensor("bqk", [P, 4], DT, kind="ExternalInput")
    woT = nc.dram_tensor("woT", [GCOLS, EMBED], DTB, kind="ExternalInput")
    maskT = nc.dram_tensor("maskT", [P, 4 * CH], DTB, kind="ExternalInput")
    onesc = nc.dram_tensor("onesc", [P, HPC], DTB, kind="ExternalInput")
    yT = nc.dram_tensor("yT", [EMBED, SEQ], DT, kind="ExternalOutput")

    with tile.TileContext(nc) as tc, ExitStack() as ctx:
        const = ctx.enter_context(tc.tile_pool(name="const", bufs=1))
        xpool = ctx.enter_context(tc.tile_pool(name="xpool", bufs=10))
        stpool = ctx.enter_context(tc.tile_pool(name="stpool", bufs=6))
        small = ctx.enter_context(tc.tile_pool(name="small", bufs=4))
        outsb = ctx.enter_context(tc.tile_pool(name="outsb", bufs=3))
        ps512 = ctx.enter_context(tc.tile_pool(name="ps512", bufs=6, space="PSUM"))
        psO = ctx.enter_context(tc.tile_pool(name="psO", bufs=2, space="PSUM"))

        # ---- persistent SBUF residents ----
        wqk_t = []
        wv_t = []
        for i in range(8):
            w1 = const.tile([P, 2 * GCOLS], DTB, tag=f"wqk{i}", name=f"wqk{i}")
            nc.sync.dma_start(out=w1, in_=wqkT[P * i:P * (i + 1), :])
            wqk_t.append(w1)
            w2 = const.tile([P, GCOLS], DTB, tag=f"wv{i}", name=f"wv{i}")
            nc.sync.dma_start(out=w2, in_=wvT[P * i:P * (i + 1), :])
            wv_t.append(w2)
        wo_t = []
        for k in range(2):
            w3 = const.tile([P, EMBED], DTB, tag=f"wo{k}", name=f"wo{k}")
            nc.sync.dma_start(out=w3, in_=woT[P * k:P * (k + 1), :])
            wo_t.append(w3)
        bqk_sb = const.tile([P, 4], DT, tag="bqk")
        nc.sync.dma_start(out=bqk_sb, in_=bqk[:])
        mask_sb = const.tile([P, 4 * CH], DTB, tag="mask")
        nc.sync.dma_start(out=mask_sb, in_=maskT[:])

        qt_t = [const.tile([P, SEQ], DTB, tag=f"qt{a}", name=f"qt{a}") for a in range(2)]
        kt_t = [const.tile([P, SEQ], DTB, tag=f"kt{a}", name=f"kt{a}") for a in range(2)]
        # v tiles: [128, 4 heads x 65] with a ones column per head at col 64
        vt_t = [const.tile([P, HPC * (HEAD + 1)], DTB, tag=f"vt{t}", name=f"vt{t}")
                for t in range(KT)]
        for t in range(KT):
            dst = vt_t[t].rearrange("p (h d) -> p h d", h=HPC)[:, :, HEAD:HEAD + 1]
            nc.sync.dma_start(out=dst, in_=onesc.rearrange("p (h o) -> p h o", o=1))
        ohat_t = [const.tile([P, SEQ], DTB, tag=f"ohat{a}", name=f"ohat{a}") for a in range(2)]

        for qi in range(NCH):
            sl = bass.ds(CH * qi, CH)
            # ---- QKV projection for this seq chunk ----
            xts = []
            for i in range(8):
                xt = xpool.tile([P, CH], DTB, tag="xt", name="xt")
                nc.sync.dma_start(out=xt, in_=xT[P * i:P * (i + 1), sl])
                xts.append(xt)
            for f in range(4):
                ps = ps512.tile([P, CH], DT, tag="ps512", name="ps512t")
                for i in range(8):
                    nc.tensor.matmul(
                        ps,
                        lhsT=wqk_t[i][:, bass.ds(P * f, P)],
                        rhs=xts[i][:],
                        start=(i == 0), stop=(i == 7),
                    )
                dst = qt_t[f] if f < 2 else kt_t[f - 2]
                nc.vector.tensor_scalar_add(dst[:, sl], ps, bqk_sb[:, f:f + 1])
            for s in range(4):
                ti = 4 * qi + s
                ps = ps512.tile([P, GCOLS], DT, tag="ps512", name="ps512v")
                for i in range(8):
                    nc.tensor.matmul(
                        ps,
                        lhsT=xts[i][:, bass.ds(P * s, P)],
                        rhs=wv_t[i][:],
                        start=(i == 0), stop=(i == 7),
                    )
                dst = vt_t[ti].rearrange("p (h d) -> p h d", h=HPC)[:, :, 0:HEAD]
                src = ps.rearrange("p (h d) -> p h d", h=HPC)
                nc.vector.tensor_copy(dst, src)

            # ---- attention for all 4 heads on this q chunk ----
            nk = 4 * qi + 4
            for hp in range(2):
                po = [psO.tile([HEAD + 1, CH], DT, tag="psO", name="psO") for _ in range(2)]
                for ki in range(nk):
                    sts = []
                    for hh in range(2):
                        h = 2 * hp + hh
                        r0 = HEAD * hh
                        ps = ps512.tile([P, CH], DT, tag="ps512", name="ps512t")
                        nc.tensor.matmul(
                            ps,
                            lhsT=kt_t[hp][r0:r0 + HEAD,
                                          bass.ds(P * ki, P)],
                            rhs=qt_t[hp][r0:r0 + HEAD, sl],
                            start=True, stop=True,
                        )
                        st = stpool.tile([P, CH], DTB, tag="st", name="st")
                        nc.scalar.activation(st[:], ps,
                                             mybir.ActivationFunctionType.Exp,
                                             scale=0.125)
                        kr = ki - 4 * qi
                        if kr >= 0:
                            nc.vector.tensor_mul(
                                st[:], st[:], mask_sb[:, bass.ds(CH * kr, CH)])
                        sts.append(st)
                    for hh in range(2):
                        h = 2 * hp + hh
                        nc.tensor.matmul(
                            po[hh],
                            lhsT=vt_t[ki][:, bass.ds((HEAD + 1) * h,
                                                     HEAD + 1)],
                            rhs=sts[hh][:],
                            start=(ki == 0), stop=(ki == nk - 1),
                        )
                for hh in range(2):
                    r0 = HEAD * hh
                    den = small.tile([1, CH], DT, tag="den", name="den")
                    nc.vector.tensor_copy(den[:], po[hh][HEAD:HEAD + 1, :])
                    recip = small.tile([1, CH], DT, tag="recip", name="recip")
                    nc.vector.reciprocal_approx_fast(recip[:], den[:])
                    recipb = small.tile([HEAD, CH], DT, tag="recipb", name="recipb")
                    nc.gpsimd.partition_broadcast(recipb[:], recip[:])
                    nc.vector.tensor_mul(ohat_t[hp][r0:r0 + HEAD, sl],
                                         po[hh][0:HEAD, :], recipb[:])

            # ---- partial output projection for this seq chunk ----
            for m in range(8):
                ps = ps512.tile([P, CH], DT, tag="ps512", name="ps512t")
                for k in range(2):
                    nc.tensor.matmul(
                        ps,
                        lhsT=wo_t[k][:, bass.ds(P * m, P)],
                        rhs=ohat_t[k][:, sl],
                        start=(k == 0), stop=(k == 1),
                    )
                ot = outsb.tile([P, CH], DT, tag="ot", name="ot")
                nc.vector.tensor_copy(ot[:], ps)
                nc.sync.dma_start(out=yT[P * m:P * (m + 1), sl], in_=ot[:])

    nc.compile()
    return nc


def _make_masks():
    m = np.zeros((P, 4 * CH), dtype=np.float32)
    p = np.arange(P)[:, None]
    c = np.arange(CH)[None, :]
    for d in range(4):
        m[:, CH * d:CH * (d + 1)] = ((p + P * d) <= c).astype(np.float32)
    return m


def kernel(x, W_qkv, b_qkv, W_out, b_out):
    global LAST_EXEC_NS, LAST_RESULTS
    x = np.asarray(x, dtype=np.float32)
    W_qkv = np.asarray(W_qkv, dtype=np.float32)
    b_qkv = np.asarray(b_qkv, dtype=np.float32)
    W_out = np.asarray(W_out, dtype=np.float32)
    b_out = np.asarray(b_out, dtype=np.float32)

    nc = _build_program()
    masks = _make_masks()

    in_maps = []
    for c in range(N_CORES):
        b, g = divmod(c, HPC)
        q0 = GCOLS * g
        wq = W_qkv[q0:q0 + GCOLS]                    # [256, 1024]
        wk = W_qkv[EMBED + q0:EMBED + q0 + GCOLS]
        wv = W_qkv[2 * EMBED + q0:2 * EMBED + q0 + GCOLS]
        bq = b_qkv[q0:q0 + GCOLS]
        bk = b_qkv[EMBED + q0:EMBED + q0 + GCOLS]
        bqk = np.stack([bq[0:P], bq[P:2 * P], bk[0:P], bk[P:2 * P]],
                       axis=1).astype(np.float32)   # [128, 4]
        in_maps.append({
            "xT": np.ascontiguousarray(x[b].T).astype(ml_dtypes.bfloat16),
            "wqkT": np.ascontiguousarray(
                np.concatenate([wq, wk], 0).T).astype(ml_dtypes.bfloat16),
            "wvT": np.ascontiguousarray(wv.T).astype(ml_dtypes.bfloat16),
            "bqk": np.ascontiguousarray(bqk),
            "woT": np.ascontiguousarray(
                W_out[:, q0:q0 + GCOLS].T).astype(ml_dtypes.bfloat16),
            "maskT": masks.astype(ml_dtypes.bfloat16),
            "onesc": np.ones((P, HPC), dtype=ml_dtypes.bfloat16),
        })

    want_trace = bool(int(os.environ.get("KTRACE", "0")))
    if want_trace:
        try:
            import antenv.axon_hooks  # noqa: F401
        except ImportError:
            want_trace = False
    res = run_bass_kernel_spmd(nc, in_maps, list(range(N_CORES)),
                               trace=want_trace,
                               tmpdir=os.environ.get("KTRACE_DIR") or None)
    LAST_EXEC_NS = res.exec_time_ns
    LAST_RESULTS = res

    out = np.empty((BATCH, SEQ, EMBED), dtype=np.float32)
    crow = (b_out + W_out @ b_qkv[2 * EMBED:]).astype(np.float32)
    for b in range(BATCH):
        acc = np.zeros((EMBED, SEQ), dtype=np.float32)
        for g in range(HPC):
            acc += res.results[HPC * b + g]["yT"]
        out[b] = acc.T + crow[None, :]
    return out
